# revision 2
# baseline (speedup 1.0000x reference)
"""DLinear fused kernel for 8 TRN2 NeuronCores.

Math: the whole module is linear in x.
  out[b,n,:] = sum_c wf_c * ( x[b,c,n,:] @ (Ws + (Wt-Ws)@A)^T ) + bias
  bias = sum(wf) * (bs + bt) + bf,  A = edge-padded moving-average matrix.

Device pipeline (per core, 8 batches = 4096 rows, 4 bb blocks of 1024):
  - x is quantized per channel to int8 on host with kappa-matched scales
    (wf_ch * s_ch == kappa), then DMA'd RAW int8 over the SP HWDGE ring:
    1 B/elem on both DMA sides (the old SWDGE cast path paid 2 B/elem on
    the SBUF-write side and was the binding resource at 66-90% queue
    busy).  kappa folds into the bf16 weights (weights-only host math).
  - channel combine runs on-chip from the int8 codes (exact in bf16):
    t = xa + xb (int8+int8->bf16), xc = t + xch (bf16+int8->bf16).
    int8 operands force DVE 1x mode (~1.07us per [128,1024] add), so the
    32 adds are statically load-balanced between DVE and Pool/GpSimd
    (Add efficiency 0.42 -> ~2.1us), which is idle now that SWDGE is
    unused: Pool takes 11 first-stage adds, DVE the other 21.
  - matmul weights-stationary bf16, k-inner per (bb, h, pc): dense 4-MM
    accumulation groups; each PSUM tile drains right after its k=3
    matmul (fused bias add on ScalarE); one wide 344 KB output DMA per
    (bb, h) leaves on the ACT HWDGE ring (3 KB rows).
  - last bb runs k-OUTER across all 6 (h, pc) PSUM tiles: after the
    final x packet only the 6 k=3 matmuls + drains remain, with per-pc
    114 KB output DMAs to shorten the tail.
DMA rings: x alone on SP HWDGE (never blocked behind another stream's
semaphore wait), weights/bias + outputs on ACT HWDGE.
"""

import numpy as np
import ml_dtypes

import concourse.bacc as bacc
import concourse.mybir as mybir
import concourse.tile as tile
from concourse.bass_utils import run_bass_kernel_spmd

N_CORES = 8
B, C, N, L, P = 64, 3, 512, 512, 336
KERNEL_W, PAD = 25, 12
BPC = B // N_CORES          # batches per core = 8
BB = 4                      # row blocks per core (1024 rows each)
NH, HW = 2, 512             # halves per block, rows per half
RB = NH * HW                # rows per block = 1024
LC = 4                      # l chunks of 128
PC, PCW = 3, 112            # p chunks x width (3*112 = 336)

BF16 = mybir.dt.bfloat16
F32 = mybir.dt.float32
I8 = mybir.dt.int8
OUT_DT = BF16

LAST_RESULT = None
_CACHE = {}

# (bb, lc) tiles whose first-stage add runs on Pool (GpSimd); chosen to
# balance DVE (1x int8 adds) against Pool (0.42-efficiency adds).
POOL_ADD1 = {t for t in range(BB * LC) if t % 3 != 2}   # 11 of 16


def _movavg_matrix():
    A = np.zeros((L, L), np.float64)
    for lp in range(L):
        for kk in range(lp - PAD, lp + PAD + 1):
            A[lp, min(max(kk, 0), L - 1)] += 1.0 / KERNEL_W
    return A


def _build():
    nc = bacc.Bacc("TRN2", target_bir_lowering=False, debug=False)
    # one transfer per (bb, lc): [128, c*1024] raw int8, 3 KB rows
    x_d = nc.dram_tensor("x", (BB, LC, 128, C * RB), I8, kind="ExternalInput")
    w_d = nc.dram_tensor("w", (LC, 128, P), BF16, kind="ExternalInput")
    b_d = nc.dram_tensor("bias", (PCW, PC), F32, kind="ExternalInput")
    # [112, pc*512] per (bb, h): 3 KB contiguous rows
    o_d = nc.dram_tensor("o", (BB, NH, PCW, PC, HW), OUT_DT, kind="ExternalOutput")

    with tile.TileContext(nc) as tc:
        with (
            tc.tile_pool(name="const", bufs=1) as constp,
            tc.tile_pool(name="xin", bufs=6) as xinp,
            tc.tile_pool(name="tp", bufs=3) as tpp,
            tc.tile_pool(name="xcp", bufs=3) as xcp,
            tc.tile_pool(name="ps", bufs=8, space="PSUM") as psp,
            tc.tile_pool(name="ostage", bufs=6) as osp,
        ):
            wts = []
            for k in range(LC):
                wt = constp.tile([128, P], BF16, tag=f"w{k}", name=f"w{k}")
                nc.scalar.dma_start(wt[:], w_d[k])
                wts.append(wt)
            btile = constp.tile([PCW, PC], F32, tag="bias", name="bias")
            nc.scalar.dma_start(btile[:], b_d[:])

            for bb in range(BB):
                last_bb = bb == BB - 1
                # ---- stream + combine ----
                xcs = []
                for lc in range(LC):
                    xf = xinp.tile([128, C * RB], I8, tag=f"x{lc}",
                                   name=f"x{lc}_{bb}")
                    nc.sync.dma_start(xf[:], x_d[bb, lc])
                    t = tpp.tile([128, RB], BF16, tag=f"t{lc}",
                                 name=f"t{lc}_{bb}")
                    eng1 = nc.gpsimd if (bb * LC + lc) in POOL_ADD1 else nc.vector
                    eng1.tensor_add(t[:], xf[:, 0:RB], xf[:, RB:2 * RB])
                    xc = xcp.tile([128, RB], BF16, tag=f"xc{lc}",
                                  name=f"xc{lc}_{bb}")
                    nc.vector.tensor_add(xc[:], t[:], xf[:, 2 * RB:3 * RB])
                    xcs.append(xc)

                # ---- matmul + drain + output ----
                if not last_bb:
                    for h in range(NH):
                        ost = osp.tile([PCW, PC * HW], OUT_DT, tag="ost",
                                       name=f"ost{bb}_{h}")
                        for pc in range(PC):
                            ps = psp.tile([PCW, HW], F32, tag="ps",
                                          name=f"ps{bb}_{h}_{pc}")
                            for k in range(LC):
                                nc.tensor.matmul(
                                    ps[:],
                                    wts[k][:, pc * PCW:(pc + 1) * PCW],
                                    xcs[k][:, h * HW:(h + 1) * HW],
                                    start=(k == 0),
                                    stop=(k == LC - 1),
                                )
                            nc.scalar.activation(
                                ost[:, pc * HW:(pc + 1) * HW],
                                ps[:],
                                mybir.ActivationFunctionType.Identity,
                                bias=btile[:, pc:pc + 1],
                            )
                        nc.scalar.dma_start(o_d[bb, h], ost[:])
                else:
                    # k-OUTER across all 6 (h, pc) tiles: only 6 matmuls +
                    # drains remain after the last x packet arrives.
                    ost = osp.tile([PCW, NH * PC * HW], OUT_DT, tag="ost",
                                   name=f"ost{bb}")
                    pss = [[psp.tile([PCW, HW], F32, tag="ps",
                                     name=f"ps{bb}_{h}_{pc}")
                            for pc in range(PC)] for h in range(NH)]
                    for k in range(LC):
                        for h in range(NH):
                            for pc in range(PC):
                                nc.tensor.matmul(
                                    pss[h][pc][:],
                                    wts[k][:, pc * PCW:(pc + 1) * PCW],
                                    xcs[k][:, h * HW:(h + 1) * HW],
                                    start=(k == 0),
                                    stop=(k == LC - 1),
                                )
                                if k == LC - 1:
                                    off = (h * PC + pc) * HW
                                    nc.scalar.activation(
                                        ost[:, off:off + HW],
                                        pss[h][pc][:],
                                        mybir.ActivationFunctionType.Identity,
                                        bias=btile[:, pc:pc + 1],
                                    )
                                    nc.scalar.dma_start(
                                        o_d[bb, h, :, pc],
                                        ost[:, off:off + HW])

    nc.compile()
    return nc


def kernel(x, Ws, bs, Wt, bt, Wf, bf):
    global LAST_RESULT
    # ---- host-side weight folding (f64, weights only) ----
    A = _movavg_matrix()
    Weff = Ws.astype(np.float64) + (Wt.astype(np.float64) - Ws.astype(np.float64)) @ A
    wf = Wf[0].astype(np.float64)                      # (3,)

    # ---- kappa-matched per-channel int8 quantization ----
    am = np.array([np.abs(x[:, ch]).max() for ch in range(C)], np.float64)
    am = np.maximum(am, 1e-30)
    kappa = float((np.abs(wf) * am).max()) / 127.0
    if kappa == 0.0:
        kappa = 1.0
    s = kappa / np.where(wf == 0, np.inf, wf)          # signed scales
    Wp = kappa * Weff                                  # (336, 512)
    WT = np.ascontiguousarray(Wp.T).reshape(LC, 128, P).astype(ml_dtypes.bfloat16)
    bias = wf.sum() * (bs.astype(np.float64) + bt.astype(np.float64)) + float(bf[0])
    bias_r = np.ascontiguousarray(bias.astype(np.float32).reshape(PC, PCW).T)

    # ---- build / compile (cached; kernel is data-independent) ----
    if "nc" not in _CACHE:
        _CACHE["nc"] = _build()
    nc = _CACHE["nc"]

    # ---- host-side quantize + sharding / layout ----
    xq = np.empty(x.shape, np.int8)
    for ch in range(C):
        xq[:, ch] = np.clip(np.round(x[:, ch] * np.float64(1.0 / s[ch])), -127, 127)
    # [core, bb, lc, p, c, h, n] -> (core, BB, LC, 128, C*1024)
    xr = xq.reshape(N_CORES, BB, NH, C, N, LC, 128)
    xr = xr.transpose(0, 1, 5, 6, 3, 2, 4)
    xr = xr.reshape(N_CORES, BB, LC, 128, C * RB)

    in_maps = []
    for i in range(N_CORES):
        in_maps.append({
            "x": np.ascontiguousarray(xr[i]),
            "w": WT,
            "bias": bias_r,
        })

    res = run_bass_kernel_spmd(nc, in_maps, core_ids=list(range(N_CORES)))
    LAST_RESULT = res

    # ---- gather / unshard ----
    outs = []
    for i in range(N_CORES):
        o = res.results[i]["o"].astype(np.float32)     # (BB, NH, 112, PC, 512)
        o = o.transpose(0, 1, 4, 3, 2).reshape(BPC, N, P)
        outs.append(o)
    out = np.stack(outs).reshape(B, N, P)[:, None]     # (64, 1, 512, 336)
    return out.astype(np.float32)


# revision 6
# speedup vs baseline: 1.1284x; 1.1284x over previous
"""DLinear fused kernel for 8 TRN2 NeuronCores.

Math: the whole module is linear in x.
  out[b,n,:] = sum_c wf_c * ( x[b,c,n,:] @ (Ws + (Wt-Ws)@A)^T ) + bias
  bias = sum(wf) * (bs + bt) + bf,  A = edge-padded moving-average matrix.

Device pipeline (per core, 8 batches = 4096 rows, 4 bb blocks of 1024):
  - x is quantized per channel to int8 on host with kappa-matched scales
    (wf_ch * s_ch == kappa); kappa folds into the bf16 weights
    (weights-only host math).  The device does the channel reduction and
    the matmul.
  - channel combine, two transports (HW-measured costs drove the mix):
      A-tiles: raw int8 over SP HWDGE (1 B/elem on both DMA sides),
        combined by two DVE mixed adds (int8 ops force 1x mode,
        ~1.14 us per [128,1024] add; Pool/GpSimd tensor ops are banned -
        measured, they ~2.5x both DVE and PE via SBUF interference).
      D-tiles: three SWDGE cast/accum DMAs (int8 HBM -> bf16 SBUF,
        accum_op=add) sum the channels inside the SDMA datapath: zero
        engine work, 2 B/elem on the SBUF-write side.  Chains are
        write-after-write serialized per bb, so hops are issued in a
        diagonal wavefront across bbs to keep the Pool sequencer from
        stalling on completion waits.
    The A/D split per bb balances DMA-queue time against DVE time.
  - matmul weights-stationary bf16, k-inner per (bb, h, pc) for the
    middle bbs; first and last bb run k-OUTER across all 6 (h, pc)
    PSUM tiles so the PE can start on partial inputs (bb0) and finish
    almost immediately after the last input packet (bb3).
  - each PSUM tile drains right after its k=3 matmul (fused bias add on
    ScalarE); wide 344 KB output DMAs (3 KB rows) per (bb, h) on the
    ACT HWDGE ring; per-pc 114 KB DMAs for bb3 to shorten the tail.
"""

import numpy as np
import ml_dtypes

import concourse.bacc as bacc
import concourse.mybir as mybir
import concourse.tile as tile
from concourse.bass_utils import run_bass_kernel_spmd

N_CORES = 8
B, C, N, L, P = 64, 3, 512, 512, 336
KERNEL_W, PAD = 25, 12
BPC = B // N_CORES          # batches per core = 8
BB = 4                      # row blocks per core (1024 rows each)
NH, HW = 2, 512             # halves per block, rows per half
RB = NH * HW                # rows per block = 1024
LC = 4                      # l chunks of 128
PC, PCW = 3, 112            # p chunks x width (3*112 = 336)

BF16 = mybir.dt.bfloat16
F32 = mybir.dt.float32
I8 = mybir.dt.int8
OUT_DT = BF16

# per-bb transport split: A = raw int8 + DVE adds, D = DMA cast/accum.
# D-slabs are capped at 2 lc (4096 B writes per partition): accum chains
# with wider per-hop writes read stale data (HW-verified failure at 6 KB).
A_LCS = {0: (0, 1, 2, 3), 1: (0, 1), 2: (0, 1), 3: (0, 1, 2)}
D_LCS = {bb: tuple(lc for lc in range(LC) if lc not in A_LCS[bb])
         for bb in range(BB)}

LAST_RESULT = None
_CACHE = {}


def _movavg_matrix():
    A = np.zeros((L, L), np.float64)
    for lp in range(L):
        for kk in range(lp - PAD, lp + PAD + 1):
            A[lp, min(max(kk, 0), L - 1)] += 1.0 / KERNEL_W
    return A


def _build():
    nc = bacc.Bacc("TRN2", target_bir_lowering=False, debug=False)
    # A-tiles: one transfer per (bb, a-slot): [128, c*1024] raw int8
    n_a_total = sum(len(v) for v in A_LCS.values())
    xa_d = nc.dram_tensor("xa", (n_a_total, 128, C * RB), I8,
                          kind="ExternalInput")
    # D-slabs: one transfer per (bb, channel): [128, nD*1024] int8
    n_d_cols = sum(len(v) for v in D_LCS.values()) * RB
    xd_d = nc.dram_tensor("xd", (C, 128, n_d_cols), I8,
                          kind="ExternalInput")
    w_d = nc.dram_tensor("w", (LC, 128, P), BF16, kind="ExternalInput")
    b_d = nc.dram_tensor("bias", (PCW, PC), F32, kind="ExternalInput")
    o_d = nc.dram_tensor("o", (BB, NH, PCW, PC, HW), OUT_DT, kind="ExternalOutput")

    # DRAM offsets per (bb) into xa / xd
    a_off = {}
    off = 0
    for bb in range(BB):
        a_off[bb] = off
        off += len(A_LCS[bb])
    d_off = {}
    off = 0
    for bb in range(BB):
        d_off[bb] = off
        off += len(D_LCS[bb]) * RB

    with tile.TileContext(nc) as tc:
        with (
            tc.tile_pool(name="const", bufs=1) as constp,
            tc.tile_pool(name="xin", bufs=2) as xinp,
            tc.tile_pool(name="tp", bufs=2) as tpp,
            tc.tile_pool(name="xcp", bufs=2) as xcp,
            tc.tile_pool(name="xd", bufs=1) as xdp,
            tc.tile_pool(name="ps", bufs=8, space="PSUM") as psp,
            tc.tile_pool(name="osw", bufs=2) as oswp,
            tc.tile_pool(name="ostage", bufs=3) as osp,
        ):
            wts = []
            for k in range(LC):
                wt = constp.tile([128, P], BF16, tag=f"w{k}", name=f"w{k}")
                nc.scalar.dma_start(wt[:], w_d[k])
                wts.append(wt)
            btile = constp.tile([PCW, PC], F32, tag="bias", name="bias")
            nc.scalar.dma_start(btile[:], b_d[:])

            # ---- D-path: cast/accum chains, diagonal wavefront issue ----
            xcd = {}
            for bb in range(BB):
                if D_LCS[bb]:
                    xcd[bb] = xdp.tile([128, len(D_LCS[bb]) * RB], BF16,
                                       tag=f"xcd{bb}", name=f"xcd{bb}")
            d_bbs = [bb for bb in range(BB) if D_LCS[bb]]
            hops = []                                   # (c, bb) wavefront
            for wave in range(C + len(d_bbs) - 1):
                for i, bb in enumerate(d_bbs):
                    c = wave - i
                    if 0 <= c < C:
                        hops.append((c, bb))
            for c, bb in hops:
                nd = len(D_LCS[bb])
                src = xd_d[c, :, d_off[bb]:d_off[bb] + nd * RB]
                if c == 0:
                    nc.gpsimd.dma_start(xcd[bb][:], src)
                else:
                    nc.gpsimd.dma_start(xcd[bb][:], src,
                                        accum_op=mybir.AluOpType.add)

            # ---- A-path stream + combine, matmul, drain ----
            def moving(bb, lc, h, xcs_a):
                if lc in A_LCS[bb]:
                    return xcs_a[lc][:, h * HW:(h + 1) * HW]
                j = D_LCS[bb].index(lc)
                return xcd[bb][:, j * RB + h * HW:j * RB + (h + 1) * HW]

            for bb in range(BB):
                xcs_a = {}
                for i, lc in enumerate(A_LCS[bb]):
                    xf = xinp.tile([128, C * RB], I8, tag=f"xa{i}",
                                   name=f"xa{i}_{bb}")
                    nc.sync.dma_start(xf[:], xa_d[a_off[bb] + i])
                    t = tpp.tile([128, RB], BF16, tag=f"t{i}",
                                 name=f"t{i}_{bb}")
                    nc.vector.tensor_add(t[:], xf[:, 0:RB], xf[:, RB:2 * RB])
                    xc = xcp.tile([128, RB], BF16, tag=f"xc{i}",
                                  name=f"xc{i}_{bb}")
                    nc.vector.tensor_add(xc[:], t[:], xf[:, 2 * RB:3 * RB])
                    xcs_a[lc] = xc

                if bb in (0, BB - 1):
                    # k-OUTER across all 6 (h, pc) tiles: bb0 starts on
                    # partial inputs, bb3 finishes right after the last one.
                    ost = oswp.tile([PCW, NH * PC * HW], OUT_DT, tag="ostw",
                                    name=f"ostw{bb}")
                    pss = [[psp.tile([PCW, HW], F32, tag="ps",
                                     name=f"ps{bb}_{h}_{pc}")
                            for pc in range(PC)] for h in range(NH)]
                    for k in range(LC):
                        for h in range(NH):
                            for pc in range(PC):
                                nc.tensor.matmul(
                                    pss[h][pc][:],
                                    wts[k][:, pc * PCW:(pc + 1) * PCW],
                                    moving(bb, k, h, xcs_a),
                                    start=(k == 0),
                                    stop=(k == LC - 1),
                                )
                                if k == LC - 1:
                                    off2 = (h * PC + pc) * HW
                                    nc.scalar.activation(
                                        ost[:, off2:off2 + HW],
                                        pss[h][pc][:],
                                        mybir.ActivationFunctionType.Identity,
                                        bias=btile[:, pc:pc + 1],
                                    )
                                    nc.scalar.dma_start(
                                        o_d[bb, h, :, pc],
                                        ost[:, off2:off2 + HW])
                else:
                    for h in range(NH):
                        ost = osp.tile([PCW, PC * HW], OUT_DT, tag="ost",
                                       name=f"ost{bb}_{h}")
                        for pc in range(PC):
                            ps = psp.tile([PCW, HW], F32, tag="ps",
                                          name=f"ps{bb}_{h}_{pc}")
                            for k in range(LC):
                                nc.tensor.matmul(
                                    ps[:],
                                    wts[k][:, pc * PCW:(pc + 1) * PCW],
                                    moving(bb, k, h, xcs_a),
                                    start=(k == 0),
                                    stop=(k == LC - 1),
                                )
                            nc.scalar.activation(
                                ost[:, pc * HW:(pc + 1) * HW],
                                ps[:],
                                mybir.ActivationFunctionType.Identity,
                                bias=btile[:, pc:pc + 1],
                            )
                        nc.scalar.dma_start(o_d[bb, h], ost[:])

    nc.compile()
    return nc


def kernel(x, Ws, bs, Wt, bt, Wf, bf):
    global LAST_RESULT
    # ---- host-side weight folding (f64, weights only) ----
    A = _movavg_matrix()
    Weff = Ws.astype(np.float64) + (Wt.astype(np.float64) - Ws.astype(np.float64)) @ A
    wf = Wf[0].astype(np.float64)                      # (3,)

    # ---- kappa-matched per-channel int8 quantization ----
    am = np.array([np.abs(x[:, ch]).max() for ch in range(C)], np.float64)
    am = np.maximum(am, 1e-30)
    kappa = float((np.abs(wf) * am).max()) / 127.0
    if kappa == 0.0:
        kappa = 1.0
    s = kappa / np.where(wf == 0, np.inf, wf)          # signed scales
    Wp = kappa * Weff                                  # (336, 512)
    WT = np.ascontiguousarray(Wp.T).reshape(LC, 128, P).astype(ml_dtypes.bfloat16)
    bias = wf.sum() * (bs.astype(np.float64) + bt.astype(np.float64)) + float(bf[0])
    bias_r = np.ascontiguousarray(bias.astype(np.float32).reshape(PC, PCW).T)

    # ---- build / compile (cached; kernel is data-independent) ----
    if "nc" not in _CACHE:
        _CACHE["nc"] = _build()
    nc = _CACHE["nc"]

    # ---- host-side quantize + sharding / layout ----
    xq = np.empty(x.shape, np.int8)
    for ch in range(C):
        xq[:, ch] = np.clip(np.round(x[:, ch] * np.float64(1.0 / s[ch])), -127, 127)
    # [core, bb, h, c, n, lc, p]
    xr = xq.reshape(N_CORES, BB, NH, C, N, LC, 128)

    n_a_total = sum(len(v) for v in A_LCS.values())
    n_d_cols = sum(len(v) for v in D_LCS.values()) * RB

    in_maps = []
    for i in range(N_CORES):
        xa = np.empty((n_a_total, 128, C * RB), np.int8)
        ai = 0
        for bb in range(BB):
            for lc in A_LCS[bb]:
                # [p, c, h, n] <- [h, c, n, p]
                blk = xr[i, bb, :, :, :, lc].transpose(3, 1, 0, 2)
                xa[ai] = blk.reshape(128, C * RB)
                ai += 1
        xd = np.empty((C, 128, n_d_cols), np.int8)
        off = 0
        for bb in range(BB):
            for lc in D_LCS[bb]:
                # [c, p, h, n] <- [h, c, n, p]
                blk = xr[i, bb, :, :, :, lc].transpose(1, 3, 0, 2)
                xd[:, :, off:off + RB] = blk.reshape(C, 128, RB)
                off += RB
        in_maps.append({
            "xa": xa,
            "xd": xd,
            "w": WT,
            "bias": bias_r,
        })

    res = run_bass_kernel_spmd(nc, in_maps, core_ids=list(range(N_CORES)))
    LAST_RESULT = res

    # ---- gather / unshard ----
    outs = []
    for i in range(N_CORES):
        o = res.results[i]["o"].astype(np.float32)     # (BB, NH, 112, PC, 512)
        o = o.transpose(0, 1, 4, 3, 2).reshape(BPC, N, P)
        outs.append(o)
    out = np.stack(outs).reshape(B, N, P)[:, None]     # (64, 1, 512, 336)
    return out.astype(np.float32)
